# revision 5
# baseline (speedup 1.0000x reference)
"""Trainium2 Bass kernel for the Fock-space shift-scale operator.

Reference math (full shapes): x = x_re + i*x_im, shape (8192, 2048) f32 each.
out[0:2, :] = 0; out[2+r, :] = x[r, :] * sqrt(r//2 + 1) for r in [0, 8190),
returned as complex64 (8192, 2048).

The scale is real, so on device the op is an elementwise multiply with a
per-row (per-partition) scalar, plus a 2-row shift handled purely by DMA
addressing. The complex64 interleave is a host-side input-marshalling choice:
each core receives its batch shard packed as (8192, 512) f32 with re/im
adjacent (the complex64 memory layout), so the device does unit-stride
tensor_scalar multiplies in place and stores rows shifted down by 2.

Sharding: data-parallel over the batch (column) axis, 2048/8 = 256 complex
columns per core. No communication.
"""

import numpy as np

import concourse.bacc as bacc
import concourse.bass as bass
import concourse.mybir as mybir
from concourse.bass_utils import run_bass_kernel_spmd
from concourse.tile import TileContext

NROWS = 8192          # 2*D
BATCH = 2048
N_CORES = 8
BCOL = BATCH // N_CORES      # 256 complex columns per core
W = 2 * BCOL                 # 512 f32 columns (interleaved re/im)
P = 128                      # SBUF partitions
NT = NROWS // P              # 64 row-tiles per core
K = 8                        # row-tiles per DMA chunk (2 MiB DMAs)
VALID_ROWS = NROWS - 2       # input rows that contribute (8190)

# Chunks of full 128-row tiles: tiles 0..55 in 7 chunks of 8, tiles 56..62 in
# one chunk of 7. Tile 63 (126 valid rows) is handled separately.
CHUNKS = [(c * K, K) for c in range(7)] + [(56, 7)]
TAIL_T = 63
TAIL_ROWS = VALID_ROWS - TAIL_T * P  # 126

_BUILT = None
LAST_RESULTS = None  # BassKernelResults of the most recent run (for test.py)


def _scale_table() -> np.ndarray:
    """(P, NT) f32: scale for input row t*128+p; rows >= 8190 get 0."""
    r = np.arange(NROWS, dtype=np.int64)
    vals = np.sqrt((r // 2 + 1).astype(np.float32))
    vals[VALID_ROWS:] = 0.0
    return np.ascontiguousarray(vals.reshape(NT, P).T)


def _pack_inputs(x_re: np.ndarray, x_im: np.ndarray) -> list[np.ndarray]:
    """Per-core (NROWS, W) f32 shards with re/im interleaved (complex64
    layout)."""
    shards = []
    for i in range(N_CORES):
        sl = slice(i * BCOL, (i + 1) * BCOL)
        packed = np.empty((NROWS, W), dtype=np.float32)
        packed[:, 0::2] = x_re[:, sl]
        packed[:, 1::2] = x_im[:, sl]
        shards.append(packed)
    return shards


def _build():
    nc = bacc.Bacc("TRN2", target_bir_lowering=False)
    x_ri = nc.dram_tensor("x_ri", [NROWS, W], mybir.dt.float32,
                          kind="ExternalInput")
    scale = nc.dram_tensor("scale", [P, NT], mybir.dt.float32,
                           kind="ExternalInput")
    out = nc.dram_tensor("out", [NROWS, W], mybir.dt.float32,
                         kind="ExternalOutput")

    with TileContext(nc) as tc:
        with (
            tc.tile_pool(name="const", bufs=1) as cpool,
            tc.tile_pool(name="io", bufs=4) as iopool,
        ):
            scale_sb = cpool.tile([P, NT], mybir.dt.float32)
            nc.sync.dma_start(out=scale_sb[:], in_=scale[:, :])

            # Output rows 0-1 are zero.
            ztile = cpool.tile([2, W], mybir.dt.float32)
            nc.vector.memset(ztile[:], 0.0)
            nc.scalar.dma_start(out=out[0:2, :], in_=ztile[:])

            for t0, nt in CHUNKS:
                start = t0 * P
                nrows = nt * P
                buf = iopool.tile([P, nt * W], mybir.dt.float32)
                din = x_ri[start:start + nrows, :].rearrange(
                    "(t p) m -> p t m", p=P)
                nc.sync.dma_start(
                    out=buf[:].rearrange("p (t m) -> p t m", t=nt), in_=din)

                for t in range(nt):
                    blk = buf[:, t * W:(t + 1) * W]
                    g = t0 + t
                    nc.vector.tensor_scalar_mul(
                        out=blk, in0=blk, scalar1=scale_sb[:, g:g + 1])

                dout = out[start + 2:start + 2 + nrows, :].rearrange(
                    "(t p) m -> p t m", p=P)
                nc.scalar.dma_start(
                    out=dout, in_=buf[:].rearrange("p (t m) -> p t m", t=nt))

            # Tail: input rows 8064..8189 (126 rows) -> output rows 8066..8191
            tstart = TAIL_T * P
            buf = iopool.tile([P, W], mybir.dt.float32)
            nc.sync.dma_start(out=buf[:TAIL_ROWS, :],
                              in_=x_ri[tstart:tstart + TAIL_ROWS, :])
            nc.vector.tensor_scalar_mul(
                out=buf[:TAIL_ROWS, :], in0=buf[:TAIL_ROWS, :],
                scalar1=scale_sb[:TAIL_ROWS, TAIL_T:TAIL_T + 1])
            nc.scalar.dma_start(out=out[tstart + 2:tstart + 2 + TAIL_ROWS, :],
                                in_=buf[:TAIL_ROWS, :])

    nc.compile()
    return nc


def benchmark(x_re: np.ndarray, x_im: np.ndarray, iters: int = 30):
    """Warm-loop timing of the NEFF execute via the same PJRT path
    run_bass_kernel_spmd uses under axon. Returns (per_iter_ns,
    zeros_only_ns, result_full) where per_iter_ns includes on-device
    zero-buffer generation + dispatch + kernel; zeros_only_ns is the same
    loop without the kernel call, for overhead subtraction."""
    import time

    import jax
    import jax.numpy as jnp
    from jax.experimental.shard_map import shard_map
    from jax.sharding import Mesh, NamedSharding, PartitionSpec

    import concourse.mybir as _mybir
    from concourse import bass2jax

    global _BUILT
    if _BUILT is None:
        _BUILT = _build()
    nc = _BUILT

    bass2jax.install_neuronx_cc_hook()

    x_re = np.asarray(x_re, dtype=np.float32)
    x_im = np.asarray(x_im, dtype=np.float32)
    scale = _scale_table()
    in_maps = [{"x_ri": s, "scale": scale}
               for s in _pack_inputs(x_re, x_im)]

    partition_name = (nc.partition_id_tensor.name
                      if nc.partition_id_tensor else None)
    in_names, out_names, out_avals, zero_shapes = [], [], [], []
    for alloc in nc.m.functions[0].allocations:
        if not isinstance(alloc, _mybir.MemoryLocationSet):
            continue
        name = alloc.memorylocations[0].name
        if alloc.kind == "ExternalInput":
            if name != partition_name:
                in_names.append(name)
        elif alloc.kind == "ExternalOutput":
            out_names.append(name)
            shape = tuple(alloc.tensor_shape)
            dtype = _mybir.dt.np(alloc.dtype)
            out_avals.append(jax.core.ShapedArray(shape, dtype))
            zero_shapes.append((shape, dtype))
    n_params = len(in_names)
    n_outs = len(out_names)
    all_in_names = in_names + out_names
    if partition_name is not None:
        all_in_names = all_in_names + [partition_name]
    donate = tuple(range(n_params, n_params + n_outs))

    def _body(*args):
        operands = list(args)
        if partition_name is not None:
            operands.append(bass2jax.partition_id_tensor())
        outs = bass2jax._bass_exec_p.bind(
            *operands,
            out_avals=tuple(out_avals),
            in_names=tuple(all_in_names),
            out_names=tuple(out_names),
            lowering_input_output_aliases=(),
            sim_require_finite=True,
            sim_require_nnan=True,
            nc=nc,
        )
        return tuple(outs)

    devices = jax.devices()[:N_CORES]
    mesh = Mesh(np.asarray(devices), ("core",))
    spec = PartitionSpec("core")
    sharded = jax.jit(
        shard_map(_body, mesh=mesh,
                  in_specs=(spec,) * (n_params + n_outs),
                  out_specs=(spec,) * n_outs,
                  check_rep=False),
        donate_argnums=donate, keep_unused=True,
    )

    sh = NamedSharding(mesh, spec)
    concat_in = [
        jax.device_put(
            np.concatenate([np.asarray(m[name]) for m in in_maps], axis=0), sh)
        for name in in_names
    ]

    def make_zeros():
        return [
            jnp.zeros((N_CORES * s[0], *s[1:]), d, device=sh)
            for (s, d) in zero_shapes
        ]

    # warm-up (compiles NEFF + caches executable)
    outs = sharded(*concat_in, *make_zeros())
    jax.block_until_ready(outs)
    result = [np.asarray(o) for o in outs]

    # zeros-only baseline
    zs = []
    t0 = time.perf_counter()
    for _ in range(iters):
        zs.append(make_zeros())
    jax.block_until_ready(zs)
    t1 = time.perf_counter()
    zeros_only_ns = (t1 - t0) / iters * 1e9
    del zs

    t0 = time.perf_counter()
    for _ in range(iters):
        outs = sharded(*concat_in, *make_zeros())
    jax.block_until_ready(outs)
    t1 = time.perf_counter()
    per_iter_ns = (t1 - t0) / iters * 1e9

    full = np.concatenate(
        [result[0].reshape(N_CORES, NROWS, W)[c].view(np.complex64)
         for c in range(N_CORES)], axis=1)
    return per_iter_ns, zeros_only_ns, full


def kernel(x_re: np.ndarray, x_im: np.ndarray) -> np.ndarray:
    global _BUILT, LAST_RESULTS
    if _BUILT is None:
        _BUILT = _build()
    nc = _BUILT

    x_re = np.asarray(x_re, dtype=np.float32)
    x_im = np.asarray(x_im, dtype=np.float32)
    scale = _scale_table()
    in_maps = [{"x_ri": s, "scale": scale}
               for s in _pack_inputs(x_re, x_im)]

    res = run_bass_kernel_spmd(nc, in_maps, core_ids=list(range(N_CORES)))
    LAST_RESULTS = res

    shards = [r["out"].view(np.complex64) for r in res.results]
    return np.concatenate(shards, axis=1)


# revision 12
# speedup vs baseline: 12.6313x; 12.6313x over previous
"""Trainium2 Bass kernel for the Fock-space shift-scale operator.

Reference math (full shapes): x = x_re + i*x_im, shape (8192, 2048) f32 each.
out[0:2, :] = 0; out[2+r, :] = x[r, :] * sqrt(r//2 + 1) for r in [0, 8190),
returned as complex64 (8192, 2048).

The scale is real, so on device the op is an elementwise multiply with a
per-row (per-partition) scalar, plus a 2-row shift handled purely by DMA
addressing. The complex64 interleave is a host-side input-marshalling choice:
each core receives its batch shard packed as (8192, 512) f32 with re/im
adjacent (the complex64 memory layout), so the device does unit-stride
tensor_scalar multiplies in place and stores rows shifted down by 2.

Sharding: data-parallel over the batch (column) axis, 2048/8 = 256 complex
columns per core. No communication.
"""

import numpy as np

import concourse.bacc as bacc
import concourse.bass as bass
import concourse.mybir as mybir
from concourse.bass_utils import run_bass_kernel_spmd
from concourse.tile import TileContext

NROWS = 8192          # 2*D
BATCH = 2048
N_CORES = 8
BCOL = BATCH // N_CORES      # 256 complex columns per core
W = 2 * BCOL                 # 512 f32 columns (interleaved re/im)
P = 128                      # SBUF partitions
NT = NROWS // P              # 64 row-tiles per core
K = 8                        # row-tiles per DMA chunk (2 MiB DMAs)
VALID_ROWS = NROWS - 2       # input rows that contribute (8190)

# Chunks of full 128-row tiles: tiles 0..55 in 7 chunks of 8, tiles 56..62 in
# one chunk of 7. Tile 63 (126 valid rows) is handled separately.
CHUNKS = [(c * K, K) for c in range(7)] + [(56, 7)]
TAIL_T = 63
TAIL_ROWS = VALID_ROWS - TAIL_T * P  # 126

_BUILT = None
LAST_RESULTS = None  # BassKernelResults of the most recent run (for test.py)


def _scale_table() -> np.ndarray:
    """(P, NT) f32: scale for input row t*128+p; rows >= 8190 get 0."""
    r = np.arange(NROWS, dtype=np.int64)
    vals = np.sqrt((r // 2 + 1).astype(np.float32))
    vals[VALID_ROWS:] = 0.0
    return np.ascontiguousarray(vals.reshape(NT, P).T)


def _pack_inputs(x_re: np.ndarray, x_im: np.ndarray) -> list[np.ndarray]:
    """Per-core (NROWS, W) f32 shards with re/im interleaved (complex64
    layout)."""
    shards = []
    for i in range(N_CORES):
        sl = slice(i * BCOL, (i + 1) * BCOL)
        packed = np.empty((NROWS, W), dtype=np.float32)
        packed[:, 0::2] = x_re[:, sl]
        packed[:, 1::2] = x_im[:, sl]
        shards.append(packed)
    return shards


def _chunks(k: int):
    """Full 128-row tiles 0..62 grouped in chunks of k; tile 63 is the tail."""
    full = NT - 1
    out, t0 = [], 0
    while t0 < full:
        nt = min(k, full - t0)
        out.append((t0, nt))
        t0 += nt
    return out


def _build(reps: int = 1, k: int = K, bufs: int = 6, split: bool = True):
    chunks = _chunks(k)
    nc = bacc.Bacc("TRN2", target_bir_lowering=False)
    x_ri = nc.dram_tensor("x_ri", [NROWS, W], mybir.dt.float32,
                          kind="ExternalInput")
    scale = nc.dram_tensor("scale", [P, NT], mybir.dt.float32,
                           kind="ExternalInput")
    out = nc.dram_tensor("out", [NROWS, W], mybir.dt.float32,
                         kind="ExternalOutput")

    with TileContext(nc) as tc:
        with (
            tc.tile_pool(name="const", bufs=1) as cpool,
            tc.tile_pool(name="io", bufs=bufs) as iopool,
        ):
            st_eng = nc.scalar if split else nc.sync
            scale_sb = cpool.tile([P, NT], mybir.dt.float32)
            nc.sync.dma_start(out=scale_sb[:], in_=scale[:, :])

            # Output rows 0-1 are zero.
            ztile = cpool.tile([2, W], mybir.dt.float32)
            nc.vector.memset(ztile[:], 0.0)
            st_eng.dma_start(out=out[0:2, :], in_=ztile[:])

            for _rep in range(reps):
                for t0, nt in chunks:
                    start = t0 * P
                    nrows = nt * P
                    buf = iopool.tile([P, nt * W], mybir.dt.float32)
                    din = x_ri[start:start + nrows, :].rearrange(
                        "(t p) m -> p t m", p=P)
                    nc.sync.dma_start(
                        out=buf[:].rearrange("p (t m) -> p t m", t=nt),
                        in_=din)

                    for t in range(nt):
                        blk = buf[:, t * W:(t + 1) * W]
                        g = t0 + t
                        nc.vector.tensor_scalar_mul(
                            out=blk, in0=blk, scalar1=scale_sb[:, g:g + 1])

                    dout = out[start + 2:start + 2 + nrows, :].rearrange(
                        "(t p) m -> p t m", p=P)
                    st_eng.dma_start(
                        out=dout,
                        in_=buf[:].rearrange("p (t m) -> p t m", t=nt))

                # Tail: input rows 8064..8189 -> output rows 8066..8191
                tstart = TAIL_T * P
                buf = iopool.tile([P, W], mybir.dt.float32)
                nc.sync.dma_start(out=buf[:TAIL_ROWS, :],
                                  in_=x_ri[tstart:tstart + TAIL_ROWS, :])
                nc.vector.tensor_scalar_mul(
                    out=buf[:TAIL_ROWS, :], in0=buf[:TAIL_ROWS, :],
                    scalar1=scale_sb[:TAIL_ROWS, TAIL_T:TAIL_T + 1])
                st_eng.dma_start(
                    out=out[tstart + 2:tstart + 2 + TAIL_ROWS, :],
                    in_=buf[:TAIL_ROWS, :])

    nc.compile()
    return nc


def rep_benchmark(x_re, x_im, reps_hi: int = 17, iters: int = 30):
    """Per-pass steady-state HW time from the dispatch-time slope between a
    1-rep NEFF and a reps_hi-rep NEFF (work unrolled inside one NEFF, so
    per-dispatch RPC overhead cancels in the slope)."""
    x_re = np.asarray(x_re, dtype=np.float32)
    x_im = np.asarray(x_im, dtype=np.float32)
    scale = _scale_table()
    in_maps = [{"x_ri": s, "scale": scale}
               for s in _pack_inputs(x_re, x_im)]
    t_lo, _ = _pjrt_timer(_build(reps=1), in_maps, iters)
    t_hi, _ = _pjrt_timer(_build(reps=reps_hi), in_maps, iters)
    return (t_hi - t_lo) / (reps_hi - 1), t_lo, t_hi


def _make_runner(nc, in_maps):
    """Build the jit(shard_map) execute path for `nc` (the same path
    run_bass_kernel_spmd uses under axon) and return (run, outs_np) where
    run(iters) times `iters` executions and returns per-iter ns, and
    outs_np() fetches the outputs of the most recent execution."""
    import time

    import jax
    import jax.numpy as jnp
    from jax.experimental.shard_map import shard_map
    from jax.sharding import Mesh, NamedSharding, PartitionSpec

    import concourse.mybir as _mybir
    from concourse import bass2jax

    bass2jax.install_neuronx_cc_hook()

    partition_name = (nc.partition_id_tensor.name
                      if nc.partition_id_tensor else None)
    in_names, out_names, out_avals, zero_shapes = [], [], [], []
    for alloc in nc.m.functions[0].allocations:
        if not isinstance(alloc, _mybir.MemoryLocationSet):
            continue
        name = alloc.memorylocations[0].name
        if alloc.kind == "ExternalInput":
            if name != partition_name:
                in_names.append(name)
        elif alloc.kind == "ExternalOutput":
            out_names.append(name)
            shape = tuple(alloc.tensor_shape)
            dtype = _mybir.dt.np(alloc.dtype)
            out_avals.append(jax.core.ShapedArray(shape, dtype))
            zero_shapes.append((shape, dtype))
    n_params = len(in_names)
    n_outs = len(out_names)
    all_in_names = in_names + out_names
    if partition_name is not None:
        all_in_names = all_in_names + [partition_name]
    donate = tuple(range(n_params, n_params + n_outs))

    def _body(*args):
        operands = list(args)
        if partition_name is not None:
            operands.append(bass2jax.partition_id_tensor())
        outs = bass2jax._bass_exec_p.bind(
            *operands,
            out_avals=tuple(out_avals),
            in_names=tuple(all_in_names),
            out_names=tuple(out_names),
            lowering_input_output_aliases=(),
            sim_require_finite=True,
            sim_require_nnan=True,
            nc=nc,
        )
        return tuple(outs)

    devices = jax.devices()[:N_CORES]
    mesh = Mesh(np.asarray(devices), ("core",))
    spec = PartitionSpec("core")
    sharded = jax.jit(
        shard_map(_body, mesh=mesh,
                  in_specs=(spec,) * (n_params + n_outs),
                  out_specs=(spec,) * n_outs,
                  check_rep=False),
        donate_argnums=donate, keep_unused=True,
    )

    sh = NamedSharding(mesh, spec)
    concat_in = [
        jax.device_put(
            np.concatenate([np.asarray(m[name]) for m in in_maps], axis=0), sh)
        for name in in_names
    ]
    make_zeros = jax.jit(
        lambda: tuple(jnp.zeros((N_CORES * s[0], *s[1:]), d)
                      for (s, d) in zero_shapes),
        out_shardings=tuple(sh for _ in zero_shapes),
    )

    state = {}

    def run(iters):
        outs = None
        t0 = time.perf_counter()
        for _ in range(iters):
            outs = sharded(*concat_in, *make_zeros())
        jax.block_until_ready(outs)
        t1 = time.perf_counter()
        state["outs"] = outs
        return (t1 - t0) / iters * 1e9

    def outs_np():
        return [np.asarray(o) for o in state["outs"]]

    run(2)  # warm-up: compiles + caches the NEFF executable
    return run, outs_np


def rep_benchmark(x_re, x_im, reps_hi: int = 17, rounds: int = 6,
                  iters: int = 10):
    """Steady-state per-pass HW time: dispatch-time slope between a 1-rep
    NEFF and a reps_hi-rep NEFF (the streaming loop unrolled inside one
    NEFF). Interleaved A/B rounds cancel the multi-ms dispatch overhead and
    its drift; returns (median_slope_ns, slopes)."""
    x_re = np.asarray(x_re, dtype=np.float32)
    x_im = np.asarray(x_im, dtype=np.float32)
    scale = _scale_table()
    in_maps = [{"x_ri": s, "scale": scale}
               for s in _pack_inputs(x_re, x_im)]
    run_lo, _ = _make_runner(_build(1), in_maps)
    run_hi, _ = _make_runner(_build(reps_hi), in_maps)
    slopes = []
    for _ in range(rounds):
        t_lo = run_lo(iters)
        t_hi = run_hi(iters)
        slopes.append((t_hi - t_lo) / (reps_hi - 1))
    slopes.sort()
    return slopes[len(slopes) // 2], slopes


def kernel(x_re: np.ndarray, x_im: np.ndarray) -> np.ndarray:
    global _BUILT, LAST_RESULTS
    if _BUILT is None:
        _BUILT = _build()
    nc = _BUILT

    x_re = np.asarray(x_re, dtype=np.float32)
    x_im = np.asarray(x_im, dtype=np.float32)
    scale = _scale_table()
    in_maps = [{"x_ri": s, "scale": scale}
               for s in _pack_inputs(x_re, x_im)]

    res = run_bass_kernel_spmd(nc, in_maps, core_ids=list(range(N_CORES)))
    LAST_RESULTS = res

    shards = [r["out"].view(np.complex64) for r in res.results]
    return np.concatenate(shards, axis=1)


# revision 13
# speedup vs baseline: 15.0778x; 1.1937x over previous
"""Trainium2 Bass kernel for the Fock-space shift-scale operator.

Reference math (full shapes): x = x_re + i*x_im, shape (8192, 2048) f32 each.
out[0:2, :] = 0; out[2+r, :] = x[r, :] * sqrt(r//2 + 1) for r in [0, 8190),
returned as complex64 (8192, 2048).

The scale is real, so on device the op is an elementwise multiply with a
per-row (per-partition) scalar, plus a 2-row shift handled purely by DMA
addressing. The complex64 interleave is a host-side input-marshalling choice:
each core receives its batch shard packed as (8192, 512) f32 with re/im
adjacent (the complex64 memory layout), so the device does unit-stride
tensor_scalar multiplies in place and stores rows shifted down by 2.

Sharding: data-parallel over the batch (column) axis, 2048/8 = 256 complex
columns per core. No communication.
"""

import numpy as np

import concourse.bacc as bacc
import concourse.bass as bass
import concourse.mybir as mybir
from concourse.bass_utils import run_bass_kernel_spmd
from concourse.tile import TileContext

NROWS = 8192          # 2*D
BATCH = 2048
N_CORES = 8
BCOL = BATCH // N_CORES      # 256 complex columns per core
W = 2 * BCOL                 # 512 f32 columns (interleaved re/im)
P = 128                      # SBUF partitions
NT = NROWS // P              # 64 row-tiles per core
K = 8                        # row-tiles per DMA chunk (2 MiB DMAs)
VALID_ROWS = NROWS - 2       # input rows that contribute (8190)

# Chunks of full 128-row tiles: tiles 0..55 in 7 chunks of 8, tiles 56..62 in
# one chunk of 7. Tile 63 (126 valid rows) is handled separately.
CHUNKS = [(c * K, K) for c in range(7)] + [(56, 7)]
TAIL_T = 63
TAIL_ROWS = VALID_ROWS - TAIL_T * P  # 126

_BUILT = None
LAST_RESULTS = None  # BassKernelResults of the most recent run (for test.py)


def _scale_table() -> np.ndarray:
    """(P, NT) f32: scale for input row t*128+p; rows >= 8190 get 0."""
    r = np.arange(NROWS, dtype=np.int64)
    vals = np.sqrt((r // 2 + 1).astype(np.float32))
    vals[VALID_ROWS:] = 0.0
    return np.ascontiguousarray(vals.reshape(NT, P).T)


def _pack_inputs(x_re: np.ndarray, x_im: np.ndarray) -> list[np.ndarray]:
    """Per-core (NROWS, W) f32 shards with re/im interleaved (complex64
    layout)."""
    shards = []
    for i in range(N_CORES):
        sl = slice(i * BCOL, (i + 1) * BCOL)
        packed = np.empty((NROWS, W), dtype=np.float32)
        packed[:, 0::2] = x_re[:, sl]
        packed[:, 1::2] = x_im[:, sl]
        shards.append(packed)
    return shards


def _chunks(k: int):
    """Full 128-row tiles 0..62 in chunks: two small leading chunks to start
    the store pipeline early, then chunks of k; tile 63 is the tail."""
    full = NT - 1
    sizes = [2, 2, 4] if k >= 4 else []
    t0 = sum(sizes)
    while t0 < full:
        nt = min(k, full - t0)
        sizes.append(nt)
        t0 += nt
    out, t0 = [], 0
    for nt in sizes:
        out.append((t0, nt))
        t0 += nt
    return out


def _build(reps: int = 1, k: int = K, bufs: int = 6, split: bool = True):
    chunks = _chunks(k)
    nc = bacc.Bacc("TRN2", target_bir_lowering=False)
    x_ri = nc.dram_tensor("x_ri", [NROWS, W], mybir.dt.float32,
                          kind="ExternalInput")
    scale = nc.dram_tensor("scale", [P, NT], mybir.dt.float32,
                           kind="ExternalInput")
    out = nc.dram_tensor("out", [NROWS, W], mybir.dt.float32,
                         kind="ExternalOutput")

    with TileContext(nc) as tc:
        with (
            tc.tile_pool(name="const", bufs=1) as cpool,
            tc.tile_pool(name="io", bufs=bufs) as iopool,
        ):
            st_eng = nc.scalar if split else nc.sync
            scale_sb = cpool.tile([P, NT], mybir.dt.float32)
            # SWDGE keeps the SP HWDGE ring free for the first input load.
            nc.gpsimd.dma_start(out=scale_sb[:], in_=scale[:, :])

            # Output rows 0-1 are zero.
            ztile = cpool.tile([2, W], mybir.dt.float32)
            nc.vector.memset(ztile[:], 0.0)
            st_eng.dma_start(out=out[0:2, :], in_=ztile[:])

            for _rep in range(reps):
                kmax = max(nt for _, nt in chunks)
                for t0, nt in chunks:
                    start = t0 * P
                    nrows = nt * P
                    buf = iopool.tile([P, kmax * W], mybir.dt.float32,
                                      name="buf")
                    din = x_ri[start:start + nrows, :].rearrange(
                        "(t p) m -> p t m", p=P)
                    nc.sync.dma_start(
                        out=buf[:, :nt * W].rearrange(
                            "p (t m) -> p t m", t=nt),
                        in_=din)

                    for t in range(nt):
                        blk = buf[:, t * W:(t + 1) * W]
                        g = t0 + t
                        nc.vector.tensor_scalar_mul(
                            out=blk, in0=blk, scalar1=scale_sb[:, g:g + 1])

                    dout = out[start + 2:start + 2 + nrows, :].rearrange(
                        "(t p) m -> p t m", p=P)
                    st_eng.dma_start(
                        out=dout,
                        in_=buf[:, :nt * W].rearrange(
                            "p (t m) -> p t m", t=nt))

                # Tail: input rows 8064..8189 -> output rows 8066..8191
                tstart = TAIL_T * P
                buf = iopool.tile([P, W], mybir.dt.float32)
                nc.sync.dma_start(out=buf[:TAIL_ROWS, :],
                                  in_=x_ri[tstart:tstart + TAIL_ROWS, :])
                nc.vector.tensor_scalar_mul(
                    out=buf[:TAIL_ROWS, :], in0=buf[:TAIL_ROWS, :],
                    scalar1=scale_sb[:TAIL_ROWS, TAIL_T:TAIL_T + 1])
                st_eng.dma_start(
                    out=out[tstart + 2:tstart + 2 + TAIL_ROWS, :],
                    in_=buf[:TAIL_ROWS, :])

    nc.compile()
    return nc


def rep_benchmark(x_re, x_im, reps_hi: int = 17, iters: int = 30):
    """Per-pass steady-state HW time from the dispatch-time slope between a
    1-rep NEFF and a reps_hi-rep NEFF (work unrolled inside one NEFF, so
    per-dispatch RPC overhead cancels in the slope)."""
    x_re = np.asarray(x_re, dtype=np.float32)
    x_im = np.asarray(x_im, dtype=np.float32)
    scale = _scale_table()
    in_maps = [{"x_ri": s, "scale": scale}
               for s in _pack_inputs(x_re, x_im)]
    t_lo, _ = _pjrt_timer(_build(reps=1), in_maps, iters)
    t_hi, _ = _pjrt_timer(_build(reps=reps_hi), in_maps, iters)
    return (t_hi - t_lo) / (reps_hi - 1), t_lo, t_hi


def _make_runner(nc, in_maps):
    """Build the jit(shard_map) execute path for `nc` (the same path
    run_bass_kernel_spmd uses under axon) and return (run, outs_np) where
    run(iters) times `iters` executions and returns per-iter ns, and
    outs_np() fetches the outputs of the most recent execution."""
    import time

    import jax
    import jax.numpy as jnp
    from jax.experimental.shard_map import shard_map
    from jax.sharding import Mesh, NamedSharding, PartitionSpec

    import concourse.mybir as _mybir
    from concourse import bass2jax

    bass2jax.install_neuronx_cc_hook()

    partition_name = (nc.partition_id_tensor.name
                      if nc.partition_id_tensor else None)
    in_names, out_names, out_avals, zero_shapes = [], [], [], []
    for alloc in nc.m.functions[0].allocations:
        if not isinstance(alloc, _mybir.MemoryLocationSet):
            continue
        name = alloc.memorylocations[0].name
        if alloc.kind == "ExternalInput":
            if name != partition_name:
                in_names.append(name)
        elif alloc.kind == "ExternalOutput":
            out_names.append(name)
            shape = tuple(alloc.tensor_shape)
            dtype = _mybir.dt.np(alloc.dtype)
            out_avals.append(jax.core.ShapedArray(shape, dtype))
            zero_shapes.append((shape, dtype))
    n_params = len(in_names)
    n_outs = len(out_names)
    all_in_names = in_names + out_names
    if partition_name is not None:
        all_in_names = all_in_names + [partition_name]
    donate = tuple(range(n_params, n_params + n_outs))

    def _body(*args):
        operands = list(args)
        if partition_name is not None:
            operands.append(bass2jax.partition_id_tensor())
        outs = bass2jax._bass_exec_p.bind(
            *operands,
            out_avals=tuple(out_avals),
            in_names=tuple(all_in_names),
            out_names=tuple(out_names),
            lowering_input_output_aliases=(),
            sim_require_finite=True,
            sim_require_nnan=True,
            nc=nc,
        )
        return tuple(outs)

    devices = jax.devices()[:N_CORES]
    mesh = Mesh(np.asarray(devices), ("core",))
    spec = PartitionSpec("core")
    sharded = jax.jit(
        shard_map(_body, mesh=mesh,
                  in_specs=(spec,) * (n_params + n_outs),
                  out_specs=(spec,) * n_outs,
                  check_rep=False),
        donate_argnums=donate, keep_unused=True,
    )

    sh = NamedSharding(mesh, spec)
    concat_in = [
        jax.device_put(
            np.concatenate([np.asarray(m[name]) for m in in_maps], axis=0), sh)
        for name in in_names
    ]
    make_zeros = jax.jit(
        lambda: tuple(jnp.zeros((N_CORES * s[0], *s[1:]), d)
                      for (s, d) in zero_shapes),
        out_shardings=tuple(sh for _ in zero_shapes),
    )

    state = {}

    def run(iters):
        outs = None
        t0 = time.perf_counter()
        for _ in range(iters):
            outs = sharded(*concat_in, *make_zeros())
        jax.block_until_ready(outs)
        t1 = time.perf_counter()
        state["outs"] = outs
        return (t1 - t0) / iters * 1e9

    def outs_np():
        return [np.asarray(o) for o in state["outs"]]

    run(2)  # warm-up: compiles + caches the NEFF executable
    return run, outs_np


def rep_benchmark(x_re, x_im, reps_hi: int = 17, rounds: int = 6,
                  iters: int = 10):
    """Steady-state per-pass HW time: dispatch-time slope between a 1-rep
    NEFF and a reps_hi-rep NEFF (the streaming loop unrolled inside one
    NEFF). Interleaved A/B rounds cancel the multi-ms dispatch overhead and
    its drift; returns (median_slope_ns, slopes)."""
    x_re = np.asarray(x_re, dtype=np.float32)
    x_im = np.asarray(x_im, dtype=np.float32)
    scale = _scale_table()
    in_maps = [{"x_ri": s, "scale": scale}
               for s in _pack_inputs(x_re, x_im)]
    run_lo, _ = _make_runner(_build(1), in_maps)
    run_hi, _ = _make_runner(_build(reps_hi), in_maps)
    slopes = []
    for _ in range(rounds):
        t_lo = run_lo(iters)
        t_hi = run_hi(iters)
        slopes.append((t_hi - t_lo) / (reps_hi - 1))
    slopes.sort()
    return slopes[len(slopes) // 2], slopes


def kernel(x_re: np.ndarray, x_im: np.ndarray) -> np.ndarray:
    global _BUILT, LAST_RESULTS
    if _BUILT is None:
        _BUILT = _build()
    nc = _BUILT

    x_re = np.asarray(x_re, dtype=np.float32)
    x_im = np.asarray(x_im, dtype=np.float32)
    scale = _scale_table()
    in_maps = [{"x_ri": s, "scale": scale}
               for s in _pack_inputs(x_re, x_im)]

    res = run_bass_kernel_spmd(nc, in_maps, core_ids=list(range(N_CORES)))
    LAST_RESULTS = res

    shards = [r["out"].view(np.complex64) for r in res.results]
    return np.concatenate(shards, axis=1)
